# revision 29
# baseline (speedup 1.0000x reference)
"""Chamfer loss (nn_ChamferLoss) on 8 Trainium2 NeuronCores.

Strategy
--------
Data-parallel over batch: bs=16 -> 2 batches per core. Per batch the device
computes the full 4096x4096 squared-distance matrix P with the TensorEngine
and reduces it in both directions:

  P[i,j] = ||g_i - p_j||^2 = a_i . b_j      (augmented inner product)
  a_i = (-2*g_i, ||g_i||^2, 1),  b_j = (p_j, 1, ||p_j||^2)   (K=5)

fp32 matmuls run at 1/4 rate on trn2, so each augmented vector is split
hi/lo into two fp16 halves (Dekker style) and the product is computed as
a single K=15 fp16 matmul (hi*hi + lo*hi + hi*lo), which runs at full PE
rate with ~fp32 accuracy (the dropped lo*lo term is ~2^-22 relative).

Per 128-row tile of P (PSUM, fp32): the ScalarEngine converts to *negated*
fp16 in SBUF (scale=-1 is free), so both min-reductions become max-
reductions that run at the DVE's 2x fp16 rate:
  - per-pred mins (partition direction): running elementwise max into C
  - per-gt mins (free direction): a TT-max fold tree; row tiles are
    processed in groups of 4 sharing one buffer so each fold level is a
    single instruction over a [128, 4, w] access pattern
Finally C is partition-max-reduced on GPSIMD (daisy chain, in quarters so
it overlaps the last folds) and summed on the ScalarEngine via accum_out;
row maxes are summed likewise. The host sums the 8 cores' partial sums.
"""

import numpy as np

import concourse.bacc as bacc
import concourse.bass_isa as bass_isa
import concourse.mybir as mybir
from concourse.bass_utils import run_bass_kernel_spmd
from concourse.tile import TileContext

N_CORES = 8
F32 = mybir.dt.float32
F16 = mybir.dt.float16


def build_nc(n_b: int, n_i: int, n_j: int):
    """Device kernel for n_b batches: inputs A [n_b,15,n_i] / B [n_b,15,n_j]
    fp16, output OUT [1, 5*n_b] fp32 holding negated partial sums of the
    per-point mins (host sums all slots and negates)."""
    assert n_i % 128 == 0 and n_j % 1024 == 0
    n_rt = n_i // 128
    psum_w = min(2048, n_j)
    n_ps = n_j // psum_w  # psum tiles per row tile
    n_mm = psum_w // 512  # matmuls per psum tile
    w_q = n_j // 4  # colsum quarter width

    nc = bacc.Bacc("TRN2", target_bir_lowering=False)
    A = nc.dram_tensor("A", [n_b, 15, n_i], F16, kind="ExternalInput")
    B = nc.dram_tensor("B", [n_b, 15, n_j], F16, kind="ExternalInput")
    OUT = nc.dram_tensor("OUT", [1, 5 * n_b], F32, kind="ExternalOutput")

    with TileContext(nc) as tc:
        with (
            tc.tile_pool(name="ab", bufs=2) as ab_pool,
            tc.tile_pool(name="psum", bufs=2, space="PSUM") as psum_pool,
            tc.tile_pool(name="ph", bufs=2) as ph_pool,
            tc.tile_pool(name="accum", bufs=2) as acc_pool,
            tc.tile_pool(name="fold", bufs=1) as fold_pool,
            tc.tile_pool(name="fin", bufs=1) as fin_pool,
        ):
            out_stats = fin_pool.tile([1, 5 * n_b], F32)
            batch_cr = []
            for b in range(n_b):
                a_sb = ab_pool.tile([15, n_i], F16, tag="a")
                b_sb = ab_pool.tile([15, n_j], F16, tag="b")
                nc.sync.dma_start(out=a_sb[:], in_=A[b])
                nc.sync.dma_start(out=b_sb[:], in_=B[b])

                C = acc_pool.tile([128, n_j], F16, tag="C")
                R = acc_pool.tile([128, n_rt], F32, tag="R")
                # graded group sizes: small first groups fill the DVE
                # pipeline sooner, then wide groups amortize op overheads
                if n_rt % 4 == 0 and n_rt >= 8:
                    groups = (
                        [1, 1, 2] + [4] * ((n_rt - 4) // 4)
                        if b == 0
                        else [4] * (n_rt // 4)
                    )
                elif n_rt % 2 == 0:
                    groups = [2] * (n_rt // 2)
                else:
                    groups = [1] * n_rt
                r_base = 0
                for G in groups:
                    split_ramp = G == 1 and n_ps == 2
                    # G row tiles share one ph buffer so each fold level
                    # is a single instruction over a [128, G, w] AP
                    if split_ramp:
                        pha = fold_pool.tile([128, psum_w], F16, tag="pha")
                        phb = fold_pool.tile([128, psum_w], F16, tag="phb")
                        ph_parts = [pha, phb]
                    else:
                        ph = ph_pool.tile([128, G * n_j], F16, tag="ph")
                    for half in range(G):
                        r = r_base + half
                        for p in range(n_ps):
                            ps = psum_pool.tile([128, psum_w], F32, tag="ps")
                            for c in range(n_mm):
                                j0 = p * psum_w + c * 512
                                nc.tensor.matmul(
                                    out=ps[:, c * 512 : (c + 1) * 512],
                                    lhsT=a_sb[:, r * 128 : (r + 1) * 128],
                                    rhs=b_sb[:, j0 : j0 + 512],
                                )
                            # convert to negated fp16 (mins become maxes)
                            if split_ramp:
                                cvt_out = ph_parts[p][:]
                            else:
                                cvt_out = ph[
                                    :,
                                    half * n_j + p * psum_w : half * n_j
                                    + (p + 1) * psum_w,
                                ]
                            nc.scalar.activation(
                                out=cvt_out,
                                in_=ps[:],
                                func=mybir.ActivationFunctionType.Copy,
                                scale=-1.0,
                            )
                    # running per-pred max (partition dir accumulates over r)
                    last_group = b == n_b - 1 and r_base + G == n_rt
                    if last_group:
                        # per-quarter tiles so each gpsimd partition-reduce
                        # starts as soon as its own quarter is final instead
                        # of after all 4 (whole-tile dep granularity); high
                        # priority front-loads these TTs before the folds
                        prio = tc.high_priority()
                        prio.__enter__()
                        for q in range(4):
                            cq = fin_pool.tile([128, w_q], F16, tag=f"CQ{q}")
                            for half in range(G):
                                phq = ph[
                                    :,
                                    half * n_j + q * w_q : half * n_j
                                    + (q + 1) * w_q,
                                ]
                                if half == 0 and r_base == 0:
                                    nc.vector.tensor_copy(out=cq[:], in_=phq)
                                elif half == 0:
                                    nc.vector.tensor_tensor(
                                        out=cq[:],
                                        in0=phq,
                                        in1=C[:, q * w_q : (q + 1) * w_q],
                                        op=mybir.AluOpType.max,
                                    )
                                else:
                                    nc.vector.tensor_tensor(
                                        out=cq[:],
                                        in0=phq,
                                        in1=cq[:],
                                        op=mybir.AluOpType.max,
                                    )
                            crq = fin_pool.tile(
                                [128, w_q], F16, tag=f"CRQ{q}"
                            )
                            nc.gpsimd.partition_all_reduce(
                                crq[:], cq[:], 128, bass_isa.ReduceOp.max
                            )
                            batch_cr.append((b, crq, 0, w_q))
                        prio.__exit__(None, None, None)
                    elif split_ramp:
                        for p in range(2):
                            tgt = C[:, p * psum_w : (p + 1) * psum_w]
                            if r_base == 0:
                                nc.vector.tensor_copy(
                                    out=tgt, in_=ph_parts[p][:]
                                )
                            else:
                                nc.vector.tensor_tensor(
                                    out=tgt,
                                    in0=ph_parts[p][:],
                                    in1=tgt,
                                    op=mybir.AluOpType.max,
                                )
                    else:
                        for half in range(G):
                            phh = ph[:, half * n_j : (half + 1) * n_j]
                            if r_base == 0 and half == 0:
                                nc.vector.tensor_copy(out=C[:], in_=phh)
                            else:
                                nc.vector.tensor_tensor(
                                    out=C[:],
                                    in0=phh,
                                    in1=C[:],
                                    op=mybir.AluOpType.max,
                                )
                    # per-gt max: grouped TT-max fold tree (2x fp16) + reduce
                    if split_ramp:
                        h0 = fold_pool.tile([128, psum_w], F16, tag="H0")
                        nc.vector.tensor_tensor(
                            out=h0[:],
                            in0=pha[:],
                            in1=phb[:],
                            op=mybir.AluOpType.max,
                        )
                        src = h0.rearrange("p (t j) -> p t j", t=1)
                        w = n_j // 4
                        lvl = 1
                    else:
                        src = ph.rearrange("p (t j) -> p t j", t=G)
                        w = n_j // 2
                        lvl = 0
                    while w >= 64:
                        dst = fold_pool.tile([128, G * w], F16, tag=f"H{lvl}")
                        dstv = dst.rearrange("p (t j) -> p t j", t=G)
                        nc.vector.tensor_tensor(
                            out=dstv[:],
                            in0=src[:, :, :w],
                            in1=src[:, :, w:],
                            op=mybir.AluOpType.max,
                        )
                        src, w, lvl = dstv, w // 2, lvl + 1
                    nc.vector.tensor_reduce(
                        out=R[:, r_base : r_base + G],
                        in_=src[:],
                        axis=mybir.AxisListType.X,
                        op=mybir.AluOpType.max,
                    )
                    r_base += G

                # partition-reduce C on gpsimd (overlaps later DVE work);
                # the DVE-side sums are deferred so the in-order DVE stream
                # never stalls waiting on gpsimd mid-kernel
                if b != n_b - 1:
                    CR = acc_pool.tile([128, n_j], F16, tag="CR")
                    for q in range(4):
                        nc.gpsimd.partition_all_reduce(
                            CR[:, q * w_q : (q + 1) * w_q],
                            C[:, q * w_q : (q + 1) * w_q],
                            128,
                            bass_isa.ReduceOp.max,
                        )
                        batch_cr.append((b, CR, q * w_q, (q + 1) * w_q))
                Rs = acc_pool.tile([128, 1], F32, tag="Rs")
                nc.vector.tensor_reduce(
                    out=Rs[:],
                    in_=R[:],
                    axis=mybir.AxisListType.X,
                    op=mybir.AluOpType.add,
                )
                # partition-sum of Rs on gpsimd; the copy into out_stats
                # runs on the scalar engine so the in-order DVE stream
                # never waits on gpsimd
                RsR = acc_pool.tile([128, 1], F32, tag="RsR")
                nc.gpsimd.partition_all_reduce(
                    RsR[:], Rs[:], 128, bass_isa.ReduceOp.add
                )
                nc.scalar.activation(
                    out=out_stats[0:1, 5 * b : 5 * b + 1],
                    in_=RsR[0:1, :],
                    func=mybir.ActivationFunctionType.Copy,
                )
            # colsum quarters on the (otherwise idle) scalar engine via
            # accum_out so the DVE stream never waits on the gpsimd reduces
            for k, (b, cr, j0, j1) in enumerate(batch_cr):
                q = k % 4
                junk = acc_pool.tile([1, w_q], F16, tag="junk")
                nc.scalar.activation(
                    out=junk[:],
                    in_=cr[0:1, j0:j1],
                    func=mybir.ActivationFunctionType.Copy,
                    accum_out=out_stats[0:1, 5 * b + 1 + q : 5 * b + 2 + q],
                )
            nc.sync.dma_start(out=OUT[:], in_=out_stats[:])
    nc.compile()
    return nc


def prep_inputs(preds: np.ndarray, gts: np.ndarray):
    """Build the K=15 fp16 hi/lo augmented operands, batched."""
    bs, n, _ = preds.shape
    g = gts.astype(np.float64)
    p = preds.astype(np.float64)
    xx = (g * g).sum(-1)  # [bs, n]
    yy = (p * p).sum(-1)
    a5 = np.stack(
        [-2 * g[..., 0], -2 * g[..., 1], -2 * g[..., 2], xx, np.ones_like(xx)], 1
    )
    b5 = np.stack([p[..., 0], p[..., 1], p[..., 2], np.ones_like(yy), yy], 1)
    a_hi = a5.astype(np.float16)
    a_lo = (a5 - a_hi.astype(np.float64)).astype(np.float16)
    b_hi = b5.astype(np.float16)
    b_lo = (b5 - b_hi.astype(np.float64)).astype(np.float16)
    A = np.concatenate([a_hi, a_lo, a_hi], 1)  # [bs, 15, n]
    B = np.concatenate([b_hi, b_hi, b_lo], 1)
    return np.ascontiguousarray(A), np.ascontiguousarray(B)


def kernel(preds: np.ndarray, gts: np.ndarray, _trace: dict | None = None):
    preds = np.asarray(preds)
    gts = np.asarray(gts)
    bs, n, _ = preds.shape
    bpc = bs // N_CORES
    A, B = prep_inputs(preds, gts)
    nc = build_nc(bpc, n, n)
    in_maps = [
        {
            "A": A[c * bpc : (c + 1) * bpc],
            "B": B[c * bpc : (c + 1) * bpc],
        }
        for c in range(N_CORES)
    ]
    kwargs = dict(_trace) if _trace else {}
    res = run_bass_kernel_spmd(nc, in_maps, core_ids=list(range(N_CORES)), **kwargs)
    total = -np.float64(
        sum(res.results[c]["OUT"].astype(np.float64).sum() for c in range(N_CORES))
    )
    if _trace is not None:
        _trace["result"] = res
    return np.float32(total / (bs * n))


# revision 30
# speedup vs baseline: 1.0073x; 1.0073x over previous
"""Chamfer loss (nn_ChamferLoss) on 8 Trainium2 NeuronCores.

Strategy
--------
Data-parallel over batch: bs=16 -> 2 batches per core. Per batch the device
computes the full 4096x4096 squared-distance matrix P with the TensorEngine
and reduces it in both directions:

  P[i,j] = ||g_i - p_j||^2 = a_i . b_j      (augmented inner product)
  a_i = (-2*g_i, ||g_i||^2, 1),  b_j = (p_j, 1, ||p_j||^2)   (K=5)

fp32 matmuls run at 1/4 rate on trn2, so each augmented vector is split
hi/lo into two fp16 halves (Dekker style) and the product is computed as
a single K=15 fp16 matmul (hi*hi + lo*hi + hi*lo), which runs at full PE
rate with ~fp32 accuracy (the dropped lo*lo term is ~2^-22 relative).

Per 128-row tile of P (PSUM, fp32): the ScalarEngine converts to *negated*
fp16 in SBUF (scale=-1 is free), so both min-reductions become max-
reductions that run at the DVE's 2x fp16 rate:
  - per-pred mins (partition direction): running elementwise max into C
  - per-gt mins (free direction): a TT-max fold tree; row tiles are
    processed in groups of 4 sharing one buffer so each fold level is a
    single instruction over a [128, 4, w] access pattern
Finally C is partition-max-reduced on GPSIMD (daisy chain, in quarters so
it overlaps the last folds) and summed on the ScalarEngine via accum_out;
row maxes are summed likewise. The host sums the 8 cores' partial sums.
"""

import numpy as np

import concourse.bacc as bacc
import concourse.bass_isa as bass_isa
import concourse.mybir as mybir
from concourse.bass_utils import run_bass_kernel_spmd
from concourse.tile import TileContext

N_CORES = 8
F32 = mybir.dt.float32
F16 = mybir.dt.float16


def build_nc(n_b: int, n_i: int, n_j: int):
    """Device kernel for n_b batches: inputs A [n_b,15,n_i] / B [n_b,15,n_j]
    fp16, output OUT [1, 5*n_b] fp32 holding negated partial sums of the
    per-point mins (host sums all slots and negates)."""
    assert n_i % 128 == 0 and n_j % 1024 == 0
    n_rt = n_i // 128
    psum_w = min(2048, n_j)
    n_ps = n_j // psum_w  # psum tiles per row tile
    n_mm = psum_w // 512  # matmuls per psum tile
    w_q = n_j // 4  # colsum quarter width

    nc = bacc.Bacc("TRN2", target_bir_lowering=False)
    A = nc.dram_tensor("A", [n_b, 15, n_i], F16, kind="ExternalInput")
    B = nc.dram_tensor("B", [n_b, 15, n_j], F16, kind="ExternalInput")
    OUT = nc.dram_tensor("OUT", [1, 5 * n_b], F32, kind="ExternalOutput")

    with TileContext(nc) as tc:
        with (
            tc.tile_pool(name="ab", bufs=2) as ab_pool,
            tc.tile_pool(name="psum", bufs=2, space="PSUM") as psum_pool,
            tc.tile_pool(name="ph", bufs=2) as ph_pool,
            tc.tile_pool(name="accum", bufs=2) as acc_pool,
            tc.tile_pool(name="fold", bufs=1) as fold_pool,
            tc.tile_pool(name="fin", bufs=1) as fin_pool,
        ):
            out_stats = fin_pool.tile([1, 5 * n_b], F32)
            batch_cr = []
            for b in range(n_b):
                a_sb = ab_pool.tile([15, n_i], F16, tag="a")
                b_sb = ab_pool.tile([15, n_j], F16, tag="b")
                nc.sync.dma_start(out=a_sb[:], in_=A[b])
                nc.sync.dma_start(out=b_sb[:], in_=B[b])

                C = acc_pool.tile([128, n_j], F16, tag="C")
                R = acc_pool.tile([128, n_rt], F32, tag="R")
                # graded group sizes: small first groups fill the DVE
                # pipeline sooner, then wide groups amortize op overheads
                if n_rt % 4 == 0 and n_rt >= 8:
                    groups = (
                        [1, 1, 2] + [4] * ((n_rt - 4) // 4)
                        if b == 0
                        else [4] * (n_rt // 4)
                    )
                elif n_rt % 2 == 0:
                    groups = [2] * (n_rt // 2)
                else:
                    groups = [1] * n_rt
                r_base = 0
                for G in groups:
                    # G row tiles share one ph buffer so each fold level
                    # is a single instruction over a [128, G, w] AP
                    ph = ph_pool.tile([128, G * n_j], F16, tag="ph")
                    for half in range(G):
                        r = r_base + half
                        for p in range(n_ps):
                            ps = psum_pool.tile([128, psum_w], F32, tag="ps")
                            for c in range(n_mm):
                                j0 = p * psum_w + c * 512
                                nc.tensor.matmul(
                                    out=ps[:, c * 512 : (c + 1) * 512],
                                    lhsT=a_sb[:, r * 128 : (r + 1) * 128],
                                    rhs=b_sb[:, j0 : j0 + 512],
                                )
                            # convert to negated fp16 (mins become maxes)
                            nc.scalar.activation(
                                out=ph[
                                    :,
                                    half * n_j + p * psum_w : half * n_j
                                    + (p + 1) * psum_w,
                                ],
                                in_=ps[:],
                                func=mybir.ActivationFunctionType.Copy,
                                scale=-1.0,
                            )
                    # running per-pred max (partition dir accumulates over r)
                    last_group = b == n_b - 1 and r_base + G == n_rt
                    if last_group:
                        # per-quarter tiles so each gpsimd partition-reduce
                        # starts as soon as its own quarter is final instead
                        # of after all 4 (whole-tile dep granularity); high
                        # priority front-loads these TTs before the folds
                        prio = tc.high_priority()
                        prio.__enter__()
                        for q in range(4):
                            cq = fin_pool.tile([128, w_q], F16, tag=f"CQ{q}")
                            for half in range(G):
                                phq = ph[
                                    :,
                                    half * n_j + q * w_q : half * n_j
                                    + (q + 1) * w_q,
                                ]
                                if half == 0 and r_base == 0:
                                    nc.vector.tensor_copy(out=cq[:], in_=phq)
                                elif half == 0:
                                    nc.vector.tensor_tensor(
                                        out=cq[:],
                                        in0=phq,
                                        in1=C[:, q * w_q : (q + 1) * w_q],
                                        op=mybir.AluOpType.max,
                                    )
                                else:
                                    nc.vector.tensor_tensor(
                                        out=cq[:],
                                        in0=phq,
                                        in1=cq[:],
                                        op=mybir.AluOpType.max,
                                    )
                            crq = fin_pool.tile(
                                [128, w_q], F16, tag=f"CRQ{q}"
                            )
                            nc.gpsimd.partition_all_reduce(
                                crq[:], cq[:], 128, bass_isa.ReduceOp.max
                            )
                            batch_cr.append((b, crq, 0, w_q))
                        prio.__exit__(None, None, None)
                    else:
                        for half in range(G):
                            phh = ph[:, half * n_j : (half + 1) * n_j]
                            if r_base == 0 and half == 0:
                                nc.vector.tensor_copy(out=C[:], in_=phh)
                            else:
                                nc.vector.tensor_tensor(
                                    out=C[:],
                                    in0=phh,
                                    in1=C[:],
                                    op=mybir.AluOpType.max,
                                )
                    # per-gt max: grouped TT-max fold tree (2x fp16) + reduce
                    src = ph.rearrange("p (t j) -> p t j", t=G)
                    w = n_j // 2
                    lvl = 0
                    while w >= 64:
                        dst = fold_pool.tile([128, G * w], F16, tag=f"H{lvl}")
                        dstv = dst.rearrange("p (t j) -> p t j", t=G)
                        nc.vector.tensor_tensor(
                            out=dstv[:],
                            in0=src[:, :, :w],
                            in1=src[:, :, w:],
                            op=mybir.AluOpType.max,
                        )
                        src, w, lvl = dstv, w // 2, lvl + 1
                    nc.vector.tensor_reduce(
                        out=R[:, r_base : r_base + G],
                        in_=src[:],
                        axis=mybir.AxisListType.X,
                        op=mybir.AluOpType.max,
                    )
                    r_base += G

                # partition-reduce C on gpsimd (overlaps later DVE work);
                # the DVE-side sums are deferred so the in-order DVE stream
                # never stalls waiting on gpsimd mid-kernel
                if b != n_b - 1:
                    CR = acc_pool.tile([128, n_j], F16, tag="CR")
                    for q in range(4):
                        nc.gpsimd.partition_all_reduce(
                            CR[:, q * w_q : (q + 1) * w_q],
                            C[:, q * w_q : (q + 1) * w_q],
                            128,
                            bass_isa.ReduceOp.max,
                        )
                        batch_cr.append((b, CR, q * w_q, (q + 1) * w_q))
                Rs = acc_pool.tile([128, 1], F32, tag="Rs")
                nc.vector.tensor_reduce(
                    out=Rs[:],
                    in_=R[:],
                    axis=mybir.AxisListType.X,
                    op=mybir.AluOpType.add,
                )
                # partition-sum of Rs on gpsimd; the copy into out_stats
                # runs on the scalar engine so the in-order DVE stream
                # never waits on gpsimd
                RsR = acc_pool.tile([128, 1], F32, tag="RsR")
                nc.gpsimd.partition_all_reduce(
                    RsR[:], Rs[:], 128, bass_isa.ReduceOp.add
                )
                nc.scalar.activation(
                    out=out_stats[0:1, 5 * b : 5 * b + 1],
                    in_=RsR[0:1, :],
                    func=mybir.ActivationFunctionType.Copy,
                )
            # colsum quarters on the (otherwise idle) scalar engine via
            # accum_out so the DVE stream never waits on the gpsimd reduces
            for k, (b, cr, j0, j1) in enumerate(batch_cr):
                q = k % 4
                junk = acc_pool.tile([1, w_q], F16, tag="junk")
                nc.scalar.activation(
                    out=junk[:],
                    in_=cr[0:1, j0:j1],
                    func=mybir.ActivationFunctionType.Copy,
                    accum_out=out_stats[0:1, 5 * b + 1 + q : 5 * b + 2 + q],
                )
            nc.sync.dma_start(out=OUT[:], in_=out_stats[:])
    nc.compile()
    return nc


def prep_inputs(preds: np.ndarray, gts: np.ndarray):
    """Build the K=15 fp16 hi/lo augmented operands, batched."""
    bs, n, _ = preds.shape
    g = gts.astype(np.float64)
    p = preds.astype(np.float64)
    xx = (g * g).sum(-1)  # [bs, n]
    yy = (p * p).sum(-1)
    a5 = np.stack(
        [-2 * g[..., 0], -2 * g[..., 1], -2 * g[..., 2], xx, np.ones_like(xx)], 1
    )
    b5 = np.stack([p[..., 0], p[..., 1], p[..., 2], np.ones_like(yy), yy], 1)
    a_hi = a5.astype(np.float16)
    a_lo = (a5 - a_hi.astype(np.float64)).astype(np.float16)
    b_hi = b5.astype(np.float16)
    b_lo = (b5 - b_hi.astype(np.float64)).astype(np.float16)
    A = np.concatenate([a_hi, a_lo, a_hi], 1)  # [bs, 15, n]
    B = np.concatenate([b_hi, b_hi, b_lo], 1)
    return np.ascontiguousarray(A), np.ascontiguousarray(B)


def kernel(preds: np.ndarray, gts: np.ndarray, _trace: dict | None = None):
    preds = np.asarray(preds)
    gts = np.asarray(gts)
    bs, n, _ = preds.shape
    bpc = bs // N_CORES
    A, B = prep_inputs(preds, gts)
    nc = build_nc(bpc, n, n)
    in_maps = [
        {
            "A": A[c * bpc : (c + 1) * bpc],
            "B": B[c * bpc : (c + 1) * bpc],
        }
        for c in range(N_CORES)
    ]
    kwargs = dict(_trace) if _trace else {}
    res = run_bass_kernel_spmd(nc, in_maps, core_ids=list(range(N_CORES)), **kwargs)
    total = -np.float64(
        sum(res.results[c]["OUT"].astype(np.float64).sum() for c in range(N_CORES))
    )
    if _trace is not None:
        _trace["result"] = res
    return np.float32(total / (bs * n))
